# revision 1
# baseline (speedup 1.0000x reference)
"""Trainium2 Bass kernel for a dense pre-norm transformer block.

Reference semantics (B=4, T=2048, D=512, H=8, DH=64, fp32):
    h  = LN(x; g, b)
    q,k,v = per-head projections of h
    att = causal softmax(q k^T / sqrt(D))
    x1 = x + (att v) @ Wproj + bproj          (heads concatenated)
    h2 = LN(x1; g, b)                         (same LN params, faithful to source)
    out = x1 + relu(h2 @ W1 + b1) @ W2 + b2

Sharding: 8 cores = 4 batches x 2 parities. Core (b, p) owns the 8
row-blocks {p, p+2, ..., p+14} (128 rows each) of batch b. Causal key
extents are rounded up to 512 so even/odd block sets see identical
work -> one uniform SPMD program, no collectives. Exact causality is
restored with multiplicative 0/1 masks on the exp() values (host
provides per-parity masks).

The dataflow needs activations feature-major (features on partitions)
for every matmul, but avoids all on-chip transposes (the DMA-transpose
path only allows 2 sync waits per instruction, which Tile's scheduler
exceeds):
  - the host passes x pre-transposed (xbT, x_ownT, bf16);
  - LN statistics are computed row-major (tokens on partitions, cheap
    free-dim reductions), written to a DRAM scratch row, and read back
    with a 0-stride partition-broadcast DMA so they can be applied in
    the transposed domain;
  - h2T is built from a transposed second projection Wproj^T @ o_catT
    plus the transposed residual, instead of transposing x1.
Scores are computed key-major [s, t]; softmax denominators come for
free from an all-ones column appended to v. All matmuls are bf16 with
fp32 PSUM accumulation; residuals, LN stats and softmax normalization
stay fp32.
"""

import os
import sys

sys.path.insert(0, "/opt/trn_rl_repo")

import numpy as np
import ml_dtypes
from contextlib import ExitStack

import concourse.bass as bass
import concourse.bacc as bacc
import concourse.mybir as mybir
import concourse.tile as tile
from concourse.bass_utils import run_bass_kernel_spmd

B, T, D, H = 4, 2048, 512, 8
DH = D // H            # 64
HID = 4 * D            # 2048
P = 128                # partitions
NT = T // P            # 16 row blocks over full T
NQ = 8                 # own row blocks per core
TQ = NQ * P            # 1024 own rows per core
EPS = 1e-5
SCALE = D ** -0.5
F32 = mybir.dt.float32
BF16 = mybir.dt.bfloat16

# first own-block (local index) attending key-block k; extents rounded to 256
JMIN = [k // 2 for k in range(16)]

_CACHED = {}


def _build_nc():
    nc = bacc.Bacc()

    xbr = nc.dram_tensor("xbr", [T, D], BF16, kind="ExternalInput")
    xbT = nc.dram_tensor("xbT", [D, T], BF16, kind="ExternalInput")
    x_own = nc.dram_tensor("x_own", [TQ, D], F32, kind="ExternalInput")
    xor_ = nc.dram_tensor("xor_", [TQ, D], BF16, kind="ExternalInput")
    x_ownT = nc.dram_tensor("x_ownT", [D, TQ], BF16, kind="ExternalInput")
    wq = nc.dram_tensor("wq", [D, D], BF16, kind="ExternalInput")
    wk = nc.dram_tensor("wk", [D, D], BF16, kind="ExternalInput")
    wv = nc.dram_tensor("wv", [D, D], BF16, kind="ExternalInput")
    wp = nc.dram_tensor("wp", [D, D], BF16, kind="ExternalInput")
    w1 = nc.dram_tensor("w1", [D, HID], BF16, kind="ExternalInput")
    w2 = nc.dram_tensor("w2", [HID, D], BF16, kind="ExternalInput")
    gvec = nc.dram_tensor("gvec", [D], F32, kind="ExternalInput")
    bvec = nc.dram_tensor("bvec", [D], F32, kind="ExternalInput")
    bpro = nc.dram_tensor("bpro", [D], F32, kind="ExternalInput")
    b1v = nc.dram_tensor("b1v", [HID], F32, kind="ExternalInput")
    b2v = nc.dram_tensor("b2v", [D], F32, kind="ExternalInput")
    masks = nc.dram_tensor("masks", [NT, P, P], BF16, kind="ExternalInput")
    out = nc.dram_tensor("out", [TQ, D], F32, kind="ExternalOutput")

    # DRAM scratch: softmax denominators + LN stat rows (for the
    # partition-broadcast round-trips)
    denbuf = nc.dram_tensor("denbuf", [H, TQ], F32)
    muv = nc.dram_tensor("muv", [T], BF16)
    rsv = nc.dram_tensor("rsv", [T], BF16)
    muov = nc.dram_tensor("muov", [TQ], BF16)
    rsov = nc.dram_tensor("rsov", [TQ], BF16)
    mu2v = nc.dram_tensor("mu2v", [TQ], BF16)
    rs2v = nc.dram_tensor("rs2v", [TQ], BF16)

    with ExitStack() as ctx:
        tc = ctx.enter_context(tile.TileContext(nc))
        consts = ctx.enter_context(tc.tile_pool(name="consts", bufs=1))

        # ---- constants ----------------------------------------------------
        g_sb = consts.tile([P, 4], F32)
        nc.sync.dma_start(out=g_sb, in_=gvec[:].rearrange("(c p) -> p c", p=P))
        b_sb = consts.tile([P, 4], F32)
        nc.sync.dma_start(out=b_sb, in_=bvec[:].rearrange("(c p) -> p c", p=P))
        eps_sb = consts.tile([P, 1], F32)
        nc.vector.memset(eps_sb, EPS)

        # ---- persistent activations --------------------------------------
        acts = ctx.enter_context(tc.tile_pool(name="acts", bufs=1))
        x1row = acts.tile([P, NQ, D], F32)
        o_catT = acts.tile([P, 4, TQ], BF16)     # (att@v)^T per head-pair
        h2T = acts.tile([P, 4, TQ], BF16)
        x_ownT_sb = acts.tile([P, 4, TQ], BF16)
        nc.sync.dma_start(
            out=x_ownT_sb, in_=x_ownT[:].rearrange("(c p) t -> p c t", p=P)
        )

        # alive through attention (phases 1-3), freed before FFN
        qkv_pool = ctx.enter_context(tc.tile_pool(name="qkv_pool", bufs=1))
        qT = qkv_pool.tile([P, 4, TQ], BF16)     # own columns only, compact
        kT = qkv_pool.tile([P, 4, T], BF16)
        v_aug = qkv_pool.tile([P, NT, H, DH + 1], BF16)   # v + ones column

        def ln_stats(x_tile, mus, vs_, it, stat_pool):
            """Row-major LN stats of x_tile [128, D] -> mu (bf16) + var col."""
            stats = stat_pool.tile([P, nc.vector.BN_STATS_DIM], F32, tag="st")
            nc.vector.bn_stats(out=stats, in_=x_tile)
            mv = stat_pool.tile([P, nc.vector.BN_AGGR_DIM], F32, tag="mv")
            nc.vector.bn_aggr(out=mv, in_=stats)
            nc.vector.tensor_copy(mus[:, it:it + 1], mv[:, 0:1])
            nc.vector.tensor_copy(vs_[:, it:it + 1], mv[:, 1:2])

        def ln_finish(vs_, rss):
            """rss (bf16) = 1/sqrt(vs_ + eps), one batched op chain."""
            nc.scalar.activation(
                out=vs_, in_=vs_,
                func=mybir.ActivationFunctionType.Sqrt,
                bias=eps_sb, scale=1.0,
            )
            nc.vector.reciprocal(out=vs_, in_=vs_)
            nc.vector.tensor_copy(rss, vs_)

        def ln_apply_T(dst, src_c, mu_b, rs_b, c):
            """dst[:,c,:] = ((src - mu)*rstd)*g + b, transposed domain."""
            nc.vector.tensor_sub(dst[:, c, :], src_c, mu_b)
            nc.vector.tensor_mul(dst[:, c, :], dst[:, c, :], rs_b)
            nc.vector.tensor_scalar(
                out=dst[:, c, :], in0=dst[:, c, :],
                scalar1=g_sb[:, c:c + 1], scalar2=b_sb[:, c:c + 1],
                op0=mybir.AluOpType.mult, op1=mybir.AluOpType.add,
            )

        # ---- phases 1+2: LN1 -> hT -> q/k/v ------------------------------
        with ExitStack() as p12:
            hT_pool = p12.enter_context(tc.tile_pool(name="hT_pool", bufs=1))
            hT = hT_pool.tile([P, 4, T], BF16)       # LN(x)^T, full batch
            hT_own = hT_pool.tile([P, 4, TQ], BF16)  # LN(x)^T, own rows
            xbT_sb = hT_pool.tile([P, 4, T], BF16)
            nc.sync.dma_start(
                out=xbT_sb, in_=xbT[:].rearrange("(c p) t -> p c t", p=P)
            )
            stat1 = p12.enter_context(tc.tile_pool(name="stat1", bufs=8))
            xpool = p12.enter_context(tc.tile_pool(name="xpool", bufs=6))
            spool = p12.enter_context(tc.tile_pool(name="spool", bufs=1))
            bpool = p12.enter_context(tc.tile_pool(name="bpool", bufs=1))

            muso = spool.tile([P, NQ], BF16)
            vso = spool.tile([P, NQ], F32)
            rsso = spool.tile([P, NQ], BF16)
            for it in range(NQ):
                x_tile = xpool.tile([P, D], BF16, tag="xr")
                nc.sync.dma_start(
                    out=x_tile, in_=xor_[it * P:(it + 1) * P, :]
                )
                ln_stats(x_tile, muso, vso, it, stat1)
            ln_finish(vso, rsso)
            nc.sync.dma_start(
                out=muov[:].rearrange("(c p) -> p c", p=P), in_=muso
            )
            nc.sync.dma_start(
                out=rsov[:].rearrange("(c p) -> p c", p=P), in_=rsso
            )
            muo_b = bpool.tile([P, TQ], BF16)
            nc.gpsimd.dma_start(out=muo_b, in_=muov[:].partition_broadcast(P))
            rso_b = bpool.tile([P, TQ], BF16)
            nc.gpsimd.dma_start(out=rso_b, in_=rsov[:].partition_broadcast(P))
            for c in range(4):
                ln_apply_T(hT_own, x_ownT_sb[:, c, :], muo_b, rso_b, c)

            wq_sb = consts.tile([P, 4, D], BF16)
            nc.sync.dma_start(
                out=wq_sb, in_=wq[:].rearrange("(c p) n -> p c n", p=P)
            )
            wk_sb = consts.tile([P, 4, D], BF16)
            nc.sync.dma_start(
                out=wk_sb, in_=wk[:].rearrange("(c p) n -> p c n", p=P)
            )
            wv_sb = consts.tile([P, 4, D], BF16)
            nc.sync.dma_start(
                out=wv_sb, in_=wv[:].rearrange("(c p) n -> p c n", p=P)
            )
            mus = spool.tile([P, NT], BF16)
            vs1 = spool.tile([P, NT], F32)
            rss = spool.tile([P, NT], BF16)
            for it in range(NT):
                xr_tile = xpool.tile([P, D], BF16, tag="xr")
                nc.sync.dma_start(
                    out=xr_tile, in_=xbr[it * P:(it + 1) * P, :]
                )
                ln_stats(xr_tile, mus, vs1, it, stat1)
            ln_finish(vs1, rss)
            nc.sync.dma_start(out=muv[:].rearrange("(c p) -> p c", p=P), in_=mus)
            nc.sync.dma_start(out=rsv[:].rearrange("(c p) -> p c", p=P), in_=rss)
            mu_b = bpool.tile([P, T], BF16)
            nc.gpsimd.dma_start(out=mu_b, in_=muv[:].partition_broadcast(P))
            rs_b = bpool.tile([P, T], BF16)
            nc.gpsimd.dma_start(out=rs_b, in_=rsv[:].partition_broadcast(P))
            for c in range(4):
                ln_apply_T(hT, xbT_sb[:, c, :], mu_b, rs_b, c)

            # ---- qT / kT / v ---------------------------------------------
            qkv_ps = p12.enter_context(
                tc.tile_pool(name="qkv_ps", bufs=4, space="PSUM")
            )
            for pair in range(4):
                for ts_ in range(2):
                    sl = slice(ts_ * 512, (ts_ + 1) * 512)
                    ps_q = qkv_ps.tile([P, 512], F32, tag="ps")
                    for c in range(4):
                        nc.tensor.matmul(
                            ps_q,
                            wq_sb[:, c, pair * P:(pair + 1) * P],
                            hT_own[:, c, sl],
                            start=(c == 0), stop=(c == 3),
                        )
                    nc.any.tensor_copy(qT[:, pair, sl], ps_q)
            for ts_ in range(4):
                sl = slice(ts_ * 512, (ts_ + 1) * 512)
                for pair in range(4):
                    ps_k = qkv_ps.tile([P, 512], F32, tag="ps")
                    for c in range(4):
                        nc.tensor.matmul(
                            ps_k,
                            wk_sb[:, c, pair * P:(pair + 1) * P],
                            hT[:, c, sl],
                            start=(c == 0), stop=(c == 3),
                        )
                    nc.any.tensor_copy(kT[:, pair, sl], ps_k)
                for st in range(4 * ts_, 4 * ts_ + 4):
                    ps_v = qkv_ps.tile([P, 512], F32, tag="ps")
                    for c in range(4):
                        nc.tensor.matmul(
                            ps_v,
                            hT[:, c, st * P:(st + 1) * P],
                            wv_sb[:, c, :],
                            start=(c == 0), stop=(c == 3),
                        )
                    nc.any.tensor_copy(
                        v_aug[:, st, :, 0:DH],
                        ps_v.rearrange("p (h e) -> p h e", h=H),
                    )
                    nc.vector.memset(v_aug[:, st, :, DH:DH + 1], 1.0)

        # ---- phase 3: attention (head pairs; scores run row-tiled
        # concurrently on the PE for the two heads of a pair) --------------
        masks_sb = consts.tile([P, NT, P], BF16)
        nc.sync.dma_start(out=masks_sb, in_=masks[:].transpose([1, 0, 2]))
        with ExitStack() as p3:
            sc_ps = p3.enter_context(
                tc.tile_pool(name="sc_ps", bufs=2, space="PSUM")
            )
            av_ps = p3.enter_context(
                tc.tile_pool(name="av_ps", bufs=4, space="PSUM")
            )
            epool = p3.enter_context(tc.tile_pool(name="epool", bufs=8))
            dpool = p3.enter_context(tc.tile_pool(name="dpool", bufs=6))
            rawhs = []
            for pair in range(4):
                prs = [slice(0, DH), slice(DH, 2 * DH)]
                oc00 = av_ps.tile([P, 512], F32, tag="oc")
                oc01 = av_ps.tile([P, 512], F32, tag="oc")
                oc10 = av_ps.tile([P, 512], F32, tag="oc")
                oc11 = av_ps.tile([P, 512], F32, tag="oc")
                ocs = [[oc00, oc01], [oc10, oc11]]   # [half][chunk]
                for k in range(NT):
                    ss = P * JMIN[k]
                    L = TQ - ss
                    sco0 = sc_ps.tile([P, 1024], F32, tag="sc")
                    sco1 = sc_ps.tile([P, 1024], F32, tag="sc")
                    scos = [sco0, sco1]
                    for half in range(2):
                        for n0 in range(0, L, 512):
                            nn = min(512, L - n0)
                            nc.tensor.matmul(
                                scos[half][:, n0:n0 + nn],
                                kT[prs[half], pair, k * P:(k + 1) * P],
                                qT[prs[half], pair, ss + n0:ss + n0 + nn],
                                start=True, stop=True,
                            )
                    for half in range(2):
                        h = 2 * pair + half
                        oc0, oc1 = ocs[half]
                        ex = epool.tile([P, 1024], BF16, tag="ex")
                        nc.scalar.activation(
                            out=ex[:, 0:L], in_=scos[half][:, 0:L],
                            func=mybir.ActivationFunctionType.Exp,
                            scale=SCALE,
                        )
                        nc.vector.tensor_mul(
                            ex[:, 0:P], ex[:, 0:P], masks_sb[:, k, :]
                        )
                        lhs_v = v_aug[:, k, h, :]
                        if ss < 512:
                            nc.tensor.matmul(
                                oc0[0:DH + 1, ss:512],
                                lhs_v,
                                ex[:, 0:512 - ss],
                                start=(k == 0), stop=(k == 7),
                            )
                            nc.tensor.matmul(
                                oc1[0:DH + 1, :],
                                lhs_v,
                                ex[:, 512 - ss:L],
                                start=(k == 0), stop=(k == 15),
                            )
                        else:
                            nc.tensor.matmul(
                                oc1[0:DH + 1, ss - 512:512],
                                lhs_v,
                                ex[:, 0:L],
                                start=False, stop=(k == 15),
                            )
                # softmax denominators -> 1/den, broadcast over 64 partitions
                # (via a DRAM round-trip; SBUF sources reject 0-stride APs)
                den2 = dpool.tile([33, TQ], F32, tag="den2", bufs=2)
                for half in range(2):
                    h = 2 * pair + half
                    oc0, oc1 = ocs[half]
                    raw = dpool.tile([DH, TQ], BF16, tag="raw", bufs=4)
                    nc.vector.tensor_copy(raw[:, 0:512], oc0[0:DH, :])
                    nc.vector.tensor_copy(raw[:, 512:TQ], oc1[0:DH, :])
                    hp = 32 * half
                    nc.vector.tensor_copy(
                        den2[hp:hp + 1, 0:512], oc0[DH:DH + 1, :]
                    )
                    nc.vector.tensor_copy(
                        den2[hp:hp + 1, 512:TQ], oc1[DH:DH + 1, :]
                    )
                    rawhs.append((h, raw))
                nc.vector.reciprocal(den2[0:1, :], den2[0:1, :])
                nc.vector.reciprocal(den2[32:33, :], den2[32:33, :])
                nc.gpsimd.dma_start(
                    out=denbuf[2 * pair, :], in_=den2[0:1, :]
                )
                nc.gpsimd.dma_start(
                    out=denbuf[2 * pair + 1, :], in_=den2[32:33, :]
                )
                for h, raw in rawhs[-2:]:
                    invb = dpool.tile([DH, TQ], F32, tag="invb", bufs=2)
                    nc.gpsimd.dma_start(
                        out=invb, in_=denbuf[h, :].partition_broadcast(DH)
                    )
                    nc.vector.tensor_mul(
                        o_catT[prs[h % 2], h // 2, :], raw, invb
                    )

        # ---- phases 4+5: proj (both orientations), residual, LN2 ---------
        wp_sb = consts.tile([P, 4, D], BF16)
        nc.sync.dma_start(
            out=wp_sb, in_=wp[:].rearrange("(c p) n -> p c n", p=P)
        )
        bpro_sb = consts.tile([P, 4], F32)
        nc.sync.dma_start(
            out=bpro_sb, in_=bpro[:].rearrange("(c p) -> p c", p=P)
        )
        bpro_bc = consts.tile([P, D], F32)
        nc.gpsimd.dma_start(out=bpro_bc, in_=bpro[:].partition_broadcast(P))
        with ExitStack() as p45:
            x1T_pool = p45.enter_context(tc.tile_pool(name="x1T_pool", bufs=1))
            x1T = x1T_pool.tile([P, 4, TQ], BF16)
            pr_ps = p45.enter_context(
                tc.tile_pool(name="pr_ps", bufs=2, space="PSUM")
            )
            prT_ps = p45.enter_context(
                tc.tile_pool(name="prT_ps", bufs=2, space="PSUM")
            )
            xopool = p45.enter_context(tc.tile_pool(name="xopool", bufs=6))
            stat2 = p45.enter_context(tc.tile_pool(name="stat2", bufs=8))
            spool2 = p45.enter_context(tc.tile_pool(name="spool2", bufs=1))
            bpool2 = p45.enter_context(tc.tile_pool(name="bpool2", bufs=1))

            mu2s = spool2.tile([P, NQ], BF16)
            vs2 = spool2.tile([P, NQ], F32)
            rs2s = spool2.tile([P, NQ], BF16)
            for tb in range(NQ):
                xo = xopool.tile([P, D], F32, tag="xo")
                nc.sync.dma_start(out=xo, in_=x_own[tb * P:(tb + 1) * P, :])
                ps = pr_ps.tile([P, D], F32, tag="pp")
                for pair in range(4):
                    nc.tensor.matmul(
                        ps,
                        o_catT[:, pair, tb * P:(tb + 1) * P],
                        wp_sb[:, pair, :],
                        start=(pair == 0), stop=(pair == 3),
                    )
                nc.vector.tensor_add(x1row[:, tb, :], ps, xo)
                nc.vector.tensor_add(x1row[:, tb, :], x1row[:, tb, :], bpro_bc)
                ln_stats(x1row[:, tb, :], mu2s, vs2, tb, stat2)
            ln_finish(vs2, rs2s)
            nc.sync.dma_start(
                out=mu2v[:].rearrange("(c p) -> p c", p=P), in_=mu2s
            )
            nc.sync.dma_start(
                out=rs2v[:].rearrange("(c p) -> p c", p=P), in_=rs2s
            )

            # transposed projection: x1T = x_ownT + Wproj^T @ o_catT + bproj
            for dt in range(4):
                for tch in range(2):
                    sl = slice(tch * 512, (tch + 1) * 512)
                    psT = prT_ps.tile([P, 512], F32, tag="pt")
                    for pair in range(4):
                        nc.tensor.matmul(
                            psT,
                            wp_sb[:, pair, dt * P:(dt + 1) * P],
                            o_catT[:, pair, sl],
                            start=(pair == 0), stop=(pair == 3),
                        )
                    nc.vector.tensor_scalar(
                        out=x1T[:, dt, sl], in0=psT,
                        scalar1=bpro_sb[:, dt:dt + 1], scalar2=None,
                        op0=mybir.AluOpType.add,
                    )
                    nc.vector.tensor_add(
                        x1T[:, dt, sl], x1T[:, dt, sl], x_ownT_sb[:, dt, sl]
                    )

            mu2_b = bpool2.tile([P, TQ], BF16)
            nc.gpsimd.dma_start(out=mu2_b, in_=mu2v[:].partition_broadcast(P))
            rs2_b = bpool2.tile([P, TQ], BF16)
            nc.gpsimd.dma_start(out=rs2_b, in_=rs2v[:].partition_broadcast(P))
            for c in range(4):
                ln_apply_T(h2T, x1T[:, c, :], mu2_b, rs2_b, c)

        # ---- phase 6: FFN + residual + store -----------------------------
        w1_sb = consts.tile([P, 4, HID], BF16)
        nc.sync.dma_start(
            out=w1_sb, in_=w1[:].rearrange("(c p) n -> p c n", p=P)
        )
        w2_sb = consts.tile([P, 16, D], BF16)
        nc.sync.dma_start(
            out=w2_sb, in_=w2[:].rearrange("(c p) n -> p c n", p=P)
        )
        b1_sb = consts.tile([P, 16], F32)
        nc.sync.dma_start(out=b1_sb, in_=b1v[:].rearrange("(c p) -> p c", p=P))
        b2_bc = consts.tile([P, D], F32)
        nc.gpsimd.dma_start(out=b2_bc, in_=b2v[:].partition_broadcast(P))
        with ExitStack() as p6:
            f1_ps = p6.enter_context(
                tc.tile_pool(name="f1_ps", bufs=3, space="PSUM")
            )
            f2_ps = p6.enter_context(
                tc.tile_pool(name="f2_ps", bufs=2, space="PSUM")
            )
            fpool = p6.enter_context(tc.tile_pool(name="fpool", bufs=18))
            opool = p6.enter_context(tc.tile_pool(name="opool", bufs=6))
            for tch in range(2):
                tsl = slice(tch * 512, (tch + 1) * 512)
                ff1 = []
                for ht in range(16):
                    ps = f1_ps.tile([P, 512], F32, tag="f1")
                    for c in range(4):
                        nc.tensor.matmul(
                            ps,
                            w1_sb[:, c, ht * P:(ht + 1) * P],
                            h2T[:, c, tsl],
                            start=(c == 0), stop=(c == 3),
                        )
                    f1s = fpool.tile([P, 512], BF16, tag="f1s")
                    # bias+relu+cast on the (otherwise idle) scalar engine
                    nc.scalar.activation(
                        out=f1s, in_=ps,
                        func=mybir.ActivationFunctionType.Relu,
                        bias=b1_sb[:, ht:ht + 1], scale=1.0,
                    )
                    ff1.append(f1s)
                for tbl in range(4):
                    tb = tch * 4 + tbl
                    ps2 = f2_ps.tile([P, D], F32, tag="f2")
                    for ht in range(16):
                        nc.tensor.matmul(
                            ps2,
                            ff1[ht][:, tbl * P:(tbl + 1) * P],
                            w2_sb[:, ht, :],
                            start=(ht == 0), stop=(ht == 15),
                        )
                    orow = opool.tile([P, D], F32, tag="or")
                    nc.vector.tensor_add(orow, ps2, x1row[:, tb, :])
                    nc.vector.tensor_add(orow, orow, b2_bc)
                    nc.sync.dma_start(
                        out=out[tb * P:(tb + 1) * P, :], in_=orow
                    )
    nc.compile()
    return nc


def _make_masks(parity: int) -> np.ndarray:
    """[NT, 128, 128] multiplicative masks for the first suffix block."""
    m = np.zeros((NT, P, P), np.float32)
    for k in range(NT):
        g = 2 * JMIN[k] + parity
        t_glob = g * P + np.arange(P)[None, :]
        s_glob = k * P + np.arange(P)[:, None]
        m[k] = (t_glob >= s_glob).astype(np.float32)
    return m.astype(ml_dtypes.bfloat16)


def _prep(inputs):
    f32 = lambda a: np.ascontiguousarray(np.asarray(a, dtype=np.float32))
    bf = lambda a: np.ascontiguousarray(
        np.asarray(a, dtype=np.float32).astype(ml_dtypes.bfloat16)
    )
    x = f32(inputs["x"])
    # [H, D, DH] -> [D, H*DH] with column h*DH+e
    wq = bf(np.asarray(inputs["Wq"], np.float32).transpose(1, 0, 2).reshape(D, D))
    wk = bf(np.asarray(inputs["Wk"], np.float32).transpose(1, 0, 2).reshape(D, D))
    wv = bf(np.asarray(inputs["Wv"], np.float32).transpose(1, 0, 2).reshape(D, D))
    common = {
        "wq": wq, "wk": wk, "wv": wv,
        "wp": bf(inputs["Wproj"]),
        "w1": bf(inputs["W1"]),
        "w2": bf(inputs["W2"]),
        "gvec": f32(inputs["ln1_g"]),
        "bvec": f32(inputs["ln1_b"]),
        "bpro": f32(inputs["bproj"]),
        "b1v": f32(inputs["b1"]),
        "b2v": f32(inputs["b2"]),
    }
    masks = [_make_masks(0), _make_masks(1)]
    in_maps = []
    for c in range(8):
        b, p = c // 2, c % 2
        xb = np.ascontiguousarray(x[b])
        xo = np.ascontiguousarray(
            x[b].reshape(NT, P, D)[p::2].reshape(TQ, D)
        )
        in_maps.append(dict(
            common,
            xor_=bf(xo),
            xbr=bf(xb),
            xbT=bf(xb.T),
            x_own=xo,
            x_ownT=bf(xo.T),
            masks=masks[p],
        ))
    return in_maps


def _run(inputs, trace=False):
    if "nc" not in _CACHED:
        _CACHED["nc"] = _build_nc()
    nc = _CACHED["nc"]
    in_maps = _prep(inputs)
    res = run_bass_kernel_spmd(nc, in_maps, core_ids=list(range(8)), trace=trace)
    out = np.empty((B, T, D), np.float32)
    for c in range(8):
        b, p = c // 2, c % 2
        out[b].reshape(NT, P, D)[p::2] = res.results[c]["out"].reshape(NQ, P, D)
    return out, res


def kernel(**inputs) -> np.ndarray:
    out, _ = _run(inputs, trace=False)
    return out



# revision 5
# speedup vs baseline: 1.1082x; 1.1082x over previous
"""Trainium2 Bass kernel for a dense pre-norm transformer block.

Reference semantics (B=4, T=2048, D=512, H=8, DH=64, fp32):
    h  = LN(x; g, b)
    q,k,v = per-head projections of h
    att = causal softmax(q k^T / sqrt(D))
    x1 = x + (att v) @ Wproj + bproj          (heads concatenated)
    h2 = LN(x1; g, b)                         (same LN params, faithful to source)
    out = x1 + relu(h2 @ W1 + b1) @ W2 + b2

Sharding: 8 cores = 4 batches x 2 parities. Core (b, p) owns the 8
row-blocks {p, p+2, ..., p+14} (128 rows each) of batch b. Causal key
extents are rounded up to 256 so even/odd block sets see identical
work -> one uniform SPMD program, no collectives. Exact causality is
restored with multiplicative 0/1 masks on the exp() values (host
provides per-parity masks and own-column slices).

v2 design (vs the first working version):
  - All large tensors are host-permuted so every DMA is one contiguous
    multi-KB descriptor per partition (128 descriptors per tensor).
  - No DRAM round-trips for LN stats or softmax denominators:
      * LN stats are computed in the transposed domain with ones-column
        matmuls (sum and sum-of-squares rows in PSUM), finished with a
        few single-partition row ops, and re-broadcast across partitions
        with rank-1 PE matmuls into PSUM (h = x*A + B with A = rstd,
        B = -mu*rstd; ln1_g is folded into Wq/Wk/Wv/W1 rows on the host).
      * softmax denominators (the free extra row from the ones-column
        appended to v) get reciprocal_approx_fast + rank-1 PE broadcast.
  - Diagonal-block causal masks are multiplied on the otherwise-idle
    GPSIMD engine; exp stays on the scalar engine (the attention-phase
    floor: ~9.4M exps at 1 elem/cycle/lane).
  - LN applies / qkv are chunk-pipelined so the PE starts ~40us earlier.
"""

import sys

sys.path.insert(0, "/opt/trn_rl_repo")

import numpy as np
import ml_dtypes
from contextlib import ExitStack

import concourse.bass as bass
import concourse.bacc as bacc
import concourse.mybir as mybir
import concourse.tile as tile
from concourse.bass_utils import run_bass_kernel_spmd

B, T, D, H = 4, 2048, 512, 8
DH = D // H            # 64
HID = 4 * D            # 2048
P = 128                # partitions
NT = T // P            # 16 row blocks over full T
NQ = 8                 # own row blocks per core
TQ = NQ * P            # 1024 own rows per core
NC = D // P            # 4 feature chunks
NH2 = HID // P         # 16
EPS = 1e-5
SCALE = D ** -0.5
F32 = mybir.dt.float32
BF16 = mybir.dt.bfloat16
AOP = mybir.AluOpType

# first own-block (local index) attending key-block k; extents rounded to 256
JMIN = [k // 2 for k in range(16)]

_CACHED = {}


def _build_nc(has_b: bool, has_bpro: bool, has_b2: bool):
    nc = bacc.Bacc()

    # host-permuted contiguous inputs: [p, c*N+n] = tensor[c*128+p, n]
    xT = nc.dram_tensor("xT", [P, NC * T], BF16, kind="ExternalInput")
    xoT = nc.dram_tensor("xoT", [P, NC * TQ], BF16, kind="ExternalInput")
    xo = nc.dram_tensor("xo", [P, NQ * D], F32, kind="ExternalInput")
    wq = nc.dram_tensor("wq", [P, NC * D], BF16, kind="ExternalInput")
    wk = nc.dram_tensor("wk", [P, NC * D], BF16, kind="ExternalInput")
    wv = nc.dram_tensor("wv", [P, NC * D], BF16, kind="ExternalInput")
    wp = nc.dram_tensor("wp", [P, NC * D], BF16, kind="ExternalInput")
    w1 = nc.dram_tensor("w1", [P, NC * HID], BF16, kind="ExternalInput")
    w2 = nc.dram_tensor("w2", [P, NH2 * D], BF16, kind="ExternalInput")
    masks = nc.dram_tensor("masks", [P, NT * P], BF16, kind="ExternalInput")
    bvec = nc.dram_tensor("bvec", [D], F32, kind="ExternalInput")
    bpro = nc.dram_tensor("bpro", [D], F32, kind="ExternalInput")
    b1v = nc.dram_tensor("b1v", [HID], F32, kind="ExternalInput")
    b2v = nc.dram_tensor("b2v", [D], F32, kind="ExternalInput")
    out = nc.dram_tensor("out", [TQ, D], F32, kind="ExternalOutput")

    with ExitStack() as ctx:
        tc = ctx.enter_context(tile.TileContext(nc))
        consts = ctx.enter_context(tc.tile_pool(name="consts", bufs=1))
        acts = ctx.enter_context(tc.tile_pool(name="acts", bufs=1))

        # ---- constants ----------------------------------------------------
        wq_sb = consts.tile([P, NC, D], BF16)
        nc.sync.dma_start(out=wq_sb, in_=wq[:].rearrange("p (c n) -> p c n", c=NC))
        wk_sb = consts.tile([P, NC, D], BF16)
        nc.sync.dma_start(out=wk_sb, in_=wk[:].rearrange("p (c n) -> p c n", c=NC))
        wv_sb = consts.tile([P, NC, D], BF16)
        nc.sync.dma_start(out=wv_sb, in_=wv[:].rearrange("p (c n) -> p c n", c=NC))
        masks_sb = consts.tile([P, NT, P], BF16)
        nc.sync.dma_start(out=masks_sb, in_=masks[:].rearrange("p (k c) -> p k c", k=NT))
        wp_sb = consts.tile([P, NC, D], BF16)
        nc.sync.dma_start(out=wp_sb, in_=wp[:].rearrange("p (c n) -> p c n", c=NC))
        w1_sb = consts.tile([P, NC, HID], BF16)
        nc.sync.dma_start(out=w1_sb, in_=w1[:].rearrange("p (c n) -> p c n", c=NC))
        w2_sb = consts.tile([P, NH2, D], BF16)
        nc.sync.dma_start(out=w2_sb, in_=w2[:].rearrange("p (c n) -> p c n", c=NH2))
        b1_sb = consts.tile([P, NH2], F32)
        nc.sync.dma_start(out=b1_sb, in_=b1v[:].rearrange("(c p) -> p c", p=P))

        onesc = consts.tile([P, 1], BF16)
        nc.vector.memset(onesc, 1.0)
        ones1 = consts.tile([1, P], F32)
        nc.vector.memset(ones1, 1.0)
        eps1 = consts.tile([1, 1], F32)
        nc.vector.memset(eps1, EPS)
        if has_b:
            b_sb = consts.tile([P, NC], F32)
            nc.sync.dma_start(out=b_sb, in_=bvec[:].rearrange("(c p) -> p c", p=P))
        if has_bpro:
            bpro_sb = consts.tile([P, NC], F32)
            nc.sync.dma_start(out=bpro_sb, in_=bpro[:].rearrange("(c p) -> p c", p=P))
            bpro_bc = consts.tile([P, D], F32)
            nc.gpsimd.dma_start(out=bpro_bc, in_=bpro[:].partition_broadcast(P))
        if has_b2:
            b2_bc = consts.tile([P, D], F32)
            nc.gpsimd.dma_start(out=b2_bc, in_=b2v[:].partition_broadcast(P))

        # ---- persistent activations --------------------------------------
        qT = acts.tile([P, NC, TQ], BF16)        # own columns, compact
        kT = acts.tile([P, NC, T], BF16)
        v_aug = acts.tile([P, NT, H, DH + 1], BF16)   # v + ones column
        o_catT = acts.tile([P, NC, TQ], BF16)
        x1T = acts.tile([P, NC, TQ], BF16)
        h2T = acts.tile([P, NC, TQ], BF16)
        x1row = acts.tile([P, NQ, D], F32)
        xoT_sb = acts.tile([P, NC, TQ], BF16)    # raw x^T, own columns
        nc.sync.dma_start(
            out=xoT_sb, in_=xoT[:].rearrange("p (c n) -> p c n", c=NC)
        )

        def ln_rows(s1, s2, t3):
            """Finish LN stats rows: s1 sum->B=-mu*rstd, s2 sumsq->A=rstd."""
            nc.vector.tensor_scalar_mul(s1, s1, 1.0 / D)      # mu
            nc.vector.tensor_scalar_mul(s2, s2, 1.0 / D)      # E[x^2]
            nc.vector.tensor_mul(t3, s1, s1)                  # mu^2
            nc.vector.tensor_sub(s2, s2, t3)                  # var
            nc.scalar.activation(
                out=s2, in_=s2,
                func=mybir.ActivationFunctionType.Sqrt,
                bias=eps1, scale=1.0,
            )
            nc.vector.reciprocal_approx_fast(out=s2, in_=s2)  # A = rstd
            nc.vector.scalar_tensor_tensor(                   # B = -mu*A
                out=s1, in0=s1, scalar=-1.0, in1=s2,
                op0=AOP.mult, op1=AOP.mult,
            )

        # ---- phase 1: LN1 (transposed-domain stats) + q/k/v --------------
        with ExitStack() as p1:
            lnp = p1.enter_context(tc.tile_pool(name="lnp", bufs=1))
            sqp = p1.enter_context(tc.tile_pool(name="sqp", bufs=2))
            rows_ps = p1.enter_context(
                tc.tile_pool(name="rows_ps", bufs=2, space="PSUM")
            )
            bc_ps = p1.enter_context(
                tc.tile_pool(name="bc_ps", bufs=4, space="PSUM")
            )
            qkv_ps = p1.enter_context(
                tc.tile_pool(name="qkv_ps", bufs=2, space="PSUM")
            )

            xT_sb = lnp.tile([P, NC, T], BF16)
            nc.sync.dma_start(
                out=xT_sb, in_=xT[:].rearrange("p (c n) -> p c n", c=NC)
            )
            hT = lnp.tile([P, NC, T], BF16)
            hoT = h2T  # reuse (h2T is not live until phase 3)
            S1 = lnp.tile([1, 4, 512], F32)    # full-T: sum -> mu -> B
            S2 = lnp.tile([1, 4, 512], F32)    # full-T: sumsq -> var -> A
            T3 = lnp.tile([1, 512], BF16)      # shared mu^2 scratch
            S1o = lnp.tile([1, 2, 512], F32)   # own: sum -> mu -> B
            S2o = lnp.tile([1, 2, 512], F32)

            # own-column stats + apply + qT (own chunks are compact)
            def emit_own_chunk(qch):
                tsl = slice(qch * 512, (qch + 1) * 512)
                sq = sqp.tile([P, NC, 512], BF16, tag="sq")
                nc.vector.tensor_mul(sq, xoT_sb[:, :, tsl], xoT_sb[:, :, tsl])
                r1 = rows_ps.tile([1, 512], F32, tag="r")
                for c in range(NC):
                    nc.tensor.matmul(
                        r1, onesc, xoT_sb[:, c, tsl],
                        start=(c == 0), stop=(c == NC - 1),
                    )
                r2 = rows_ps.tile([1, 512], F32, tag="r")
                for c in range(NC):
                    nc.tensor.matmul(
                        r2, onesc, sq[:, c, :],
                        start=(c == 0), stop=(c == NC - 1),
                    )
                s1, s2, t3 = S1o[0:1, qch, :], S2o[0:1, qch, :], T3[0:1, :]
                nc.vector.tensor_copy(s1, r1)
                nc.vector.tensor_copy(s2, r2)
                ln_rows(s1, s2, t3)
                a_ps = bc_ps.tile([P, 512], F32, tag="bc")
                nc.tensor.matmul(a_ps, ones1, s2, start=True, stop=True)
                b_ps = bc_ps.tile([P, 512], F32, tag="bc")
                nc.tensor.matmul(b_ps, ones1, s1, start=True, stop=True)
                for c in range(NC):
                    nc.vector.tensor_mul(hoT[:, c, tsl], xoT_sb[:, c, tsl], a_ps)
                    nc.vector.tensor_add(hoT[:, c, tsl], hoT[:, c, tsl], b_ps)
                    if has_b:
                        nc.gpsimd.tensor_scalar_add(
                            hoT[:, c, tsl], hoT[:, c, tsl], b_sb[:, c:c + 1]
                        )
                for pair in range(4):
                    ps_q = qkv_ps.tile([P, 512], F32, tag="ps")
                    for c in range(NC):
                        nc.tensor.matmul(
                            ps_q,
                            wq_sb[:, c, pair * P:(pair + 1) * P],
                            hoT[:, c, tsl],
                            start=(c == 0), stop=(c == NC - 1),
                        )
                    nc.any.tensor_copy(qT[:, pair, tsl], ps_q)

            emit_own_chunk(0)
            emit_own_chunk(1)

            for tch in range(4):
                tsl = slice(tch * 512, (tch + 1) * 512)
                sq = sqp.tile([P, NC, 512], BF16, tag="sq")
                nc.vector.tensor_mul(sq, xT_sb[:, :, tsl], xT_sb[:, :, tsl])
                r1 = rows_ps.tile([1, 512], F32, tag="r")
                for c in range(NC):
                    nc.tensor.matmul(
                        r1, onesc, xT_sb[:, c, tsl],
                        start=(c == 0), stop=(c == NC - 1),
                    )
                r2 = rows_ps.tile([1, 512], F32, tag="r")
                for c in range(NC):
                    nc.tensor.matmul(
                        r2, onesc, sq[:, c, :],
                        start=(c == 0), stop=(c == NC - 1),
                    )
                s1, s2, t3 = S1[0:1, tch, :], S2[0:1, tch, :], T3[0:1, :]
                nc.vector.tensor_copy(s1, r1)
                nc.vector.tensor_copy(s2, r2)
                ln_rows(s1, s2, t3)
                a_ps = bc_ps.tile([P, 512], F32, tag="bc")
                nc.tensor.matmul(a_ps, ones1, s2, start=True, stop=True)
                b_ps = bc_ps.tile([P, 512], F32, tag="bc")
                nc.tensor.matmul(b_ps, ones1, s1, start=True, stop=True)
                for c in range(NC):
                    nc.vector.tensor_mul(hT[:, c, tsl], xT_sb[:, c, tsl], a_ps)
                    nc.vector.tensor_add(hT[:, c, tsl], hT[:, c, tsl], b_ps)
                    if has_b:
                        nc.gpsimd.tensor_scalar_add(
                            hT[:, c, tsl], hT[:, c, tsl], b_sb[:, c:c + 1]
                        )
                # kT / v for this chunk
                for pair in range(4):
                    ps_k = qkv_ps.tile([P, 512], F32, tag="ps")
                    for c in range(NC):
                        nc.tensor.matmul(
                            ps_k,
                            wk_sb[:, c, pair * P:(pair + 1) * P],
                            hT[:, c, tsl],
                            start=(c == 0), stop=(c == NC - 1),
                        )
                    nc.any.tensor_copy(kT[:, pair, tsl], ps_k)
                for st in range(4 * tch, 4 * tch + 4):
                    ps_v = qkv_ps.tile([P, 512], F32, tag="ps")
                    for c in range(NC):
                        nc.tensor.matmul(
                            ps_v,
                            hT[:, c, st * P:(st + 1) * P],
                            wv_sb[:, c, :],
                            start=(c == 0), stop=(c == NC - 1),
                        )
                    nc.any.tensor_copy(
                        v_aug[:, st, :, 0:DH],
                        ps_v.rearrange("p (h e) -> p h e", h=H),
                    )
                    nc.vector.memset(v_aug[:, st, :, DH:DH + 1], 1.0)

        # ---- phase 2: attention ------------------------------------------
        prs = [slice(0, DH), slice(DH, 2 * DH)]
        with ExitStack() as p2:
            sc_ps = p2.enter_context(
                tc.tile_pool(name="sc_ps", bufs=2, space="PSUM")
            )
            av_ps = p2.enter_context(
                tc.tile_pool(name="av_ps", bufs=4, space="PSUM")
            )
            epool = p2.enter_context(tc.tile_pool(name="epool", bufs=6))
            rpool = p2.enter_context(tc.tile_pool(name="rpool", bufs=4))
            dpool = p2.enter_context(tc.tile_pool(name="dpool", bufs=2))
            for pair in range(4):
                oc00 = av_ps.tile([P, 512], F32, tag="oc")
                oc01 = av_ps.tile([P, 512], F32, tag="oc")
                oc10 = av_ps.tile([P, 512], F32, tag="oc")
                oc11 = av_ps.tile([P, 512], F32, tag="oc")
                ocs = [[oc00, oc01], [oc10, oc11]]   # [half][chunk]
                for k in range(NT):
                    ss = P * JMIN[k]
                    L = TQ - ss
                    sco0 = sc_ps.tile([P, 1024], F32, tag="sc")
                    sco1 = sc_ps.tile([P, 1024], F32, tag="sc")
                    scos = [sco0, sco1]
                    for half in range(2):
                        for n0 in range(0, L, 512):
                            nn = min(512, L - n0)
                            nc.tensor.matmul(
                                scos[half][:, n0:n0 + nn],
                                kT[prs[half], pair, k * P:(k + 1) * P],
                                qT[prs[half], pair, ss + n0:ss + n0 + nn],
                                start=True, stop=True,
                            )
                    for half in range(2):
                        h = 2 * pair + half
                        oc0, oc1 = ocs[half]
                        ex = epool.tile([P, 1024], BF16, tag="ex")
                        nc.scalar.activation(
                            out=ex[:, 0:L], in_=scos[half][:, 0:L],
                            func=mybir.ActivationFunctionType.Exp,
                            scale=SCALE,
                        )
                        nc.gpsimd.tensor_mul(
                            ex[:, 0:P], ex[:, 0:P], masks_sb[:, k, :]
                        )
                        lhs_v = v_aug[:, k, h, :]
                        if ss < 512:
                            nc.tensor.matmul(
                                oc0[0:DH + 1, ss:512],
                                lhs_v,
                                ex[:, 0:512 - ss],
                                start=(k == 0), stop=(k == 7),
                            )
                            nc.tensor.matmul(
                                oc1[0:DH + 1, :],
                                lhs_v,
                                ex[:, 512 - ss:L],
                                start=(k == 0), stop=(k == 15),
                            )
                        else:
                            nc.tensor.matmul(
                                oc1[0:DH + 1, ss - 512:512],
                                lhs_v,
                                ex[:, 0:L],
                                start=False, stop=(k == 15),
                            )
                # denominators -> 1/den -> rank-1 PE broadcast -> normalize
                den = dpool.tile([1, 2, TQ], F32, tag="den")
                raws = []
                for half in range(2):
                    oc0, oc1 = ocs[half]
                    raw = rpool.tile([DH, TQ], BF16, tag="raw")
                    nc.vector.tensor_copy(raw[:, 0:512], oc0[0:DH, :])
                    nc.vector.tensor_copy(raw[:, 512:TQ], oc1[0:DH, :])
                    nc.vector.tensor_copy(den[0:1, half, 0:512], oc0[DH:DH + 1, :])
                    nc.vector.tensor_copy(den[0:1, half, 512:TQ], oc1[DH:DH + 1, :])
                    raws.append(raw)
                nc.vector.reciprocal_approx_fast(out=den, in_=den)
                ib_ps = sc_ps.tile([P, 1024], F32, tag="sc")
                for half in range(2):
                    for ch in range(2):
                        nc.tensor.matmul(
                            ib_ps[64 * half:64 * half + DH,
                                  ch * 512:(ch + 1) * 512],
                            ones1[0:1, 0:DH],
                            den[0:1, half, ch * 512:(ch + 1) * 512],
                            start=True, stop=True,
                        )
                for half in range(2):
                    nc.vector.tensor_mul(
                        o_catT[prs[half], pair, :], raws[half],
                        ib_ps[64 * half:64 * half + DH, :],
                    )

        # ---- phase 3: proj (both orientations), residual, LN2 -------------
        with ExitStack() as p3:
            pr_ps = p3.enter_context(
                tc.tile_pool(name="pr_ps", bufs=2, space="PSUM")
            )
            prT_ps = p3.enter_context(
                tc.tile_pool(name="prT_ps", bufs=2, space="PSUM")
            )
            rows2_ps = p3.enter_context(
                tc.tile_pool(name="rows2_ps", bufs=2, space="PSUM")
            )
            bc2_ps = p3.enter_context(
                tc.tile_pool(name="bc2_ps", bufs=2, space="PSUM")
            )
            xopool = p3.enter_context(tc.tile_pool(name="xopool", bufs=4))
            sq2p = p3.enter_context(tc.tile_pool(name="sq2p", bufs=2))
            ln2p = p3.enter_context(tc.tile_pool(name="ln2p", bufs=1))

            # row-major proj + residual
            for tb in range(NQ):
                xo_t = xopool.tile([P, D], F32, tag="xo")
                nc.sync.dma_start(
                    out=xo_t,
                    in_=xo[:].rearrange("p (a b) -> p a b", a=NQ)[:, tb, :],
                )
                ps = pr_ps.tile([P, D], F32, tag="pp")
                for pair in range(4):
                    nc.tensor.matmul(
                        ps,
                        o_catT[:, pair, tb * P:(tb + 1) * P],
                        wp_sb[:, pair, :],
                        start=(pair == 0), stop=(pair == 3),
                    )
                nc.vector.tensor_add(x1row[:, tb, :], ps, xo_t)
                if has_bpro:
                    nc.vector.tensor_add(
                        x1row[:, tb, :], x1row[:, tb, :], bpro_bc
                    )

            # transposed proj: x1T = xoT + Wproj^T @ o_catT (+ bproj)
            for dt in range(NC):
                for tch in range(2):
                    sl = slice(tch * 512, (tch + 1) * 512)
                    psT = prT_ps.tile([P, 512], F32, tag="pt")
                    for pair in range(4):
                        nc.tensor.matmul(
                            psT,
                            wp_sb[:, pair, dt * P:(dt + 1) * P],
                            o_catT[:, pair, sl],
                            start=(pair == 0), stop=(pair == 3),
                        )
                    nc.vector.tensor_add(x1T[:, dt, sl], psT, xoT_sb[:, dt, sl])
                    if has_bpro:
                        nc.vector.tensor_scalar_add(
                            x1T[:, dt, sl], x1T[:, dt, sl], bpro_sb[:, dt:dt + 1]
                        )

            # LN2 stats from x1T + apply -> h2T
            S1b = ln2p.tile([1, 2, 512], F32)
            S2b = ln2p.tile([1, 2, 512], F32)
            T3b = ln2p.tile([1, 2, 512], BF16)
            for tch in range(2):
                tsl = slice(tch * 512, (tch + 1) * 512)
                sq2 = sq2p.tile([P, NC, 512], BF16, tag="sq2")
                nc.vector.tensor_mul(sq2, x1T[:, :, tsl], x1T[:, :, tsl])
                r1 = rows2_ps.tile([1, 512], F32, tag="r")
                for c in range(NC):
                    nc.tensor.matmul(
                        r1, onesc, x1T[:, c, tsl],
                        start=(c == 0), stop=(c == NC - 1),
                    )
                r2 = rows2_ps.tile([1, 512], F32, tag="r")
                for c in range(NC):
                    nc.tensor.matmul(
                        r2, onesc, sq2[:, c, :],
                        start=(c == 0), stop=(c == NC - 1),
                    )
                s1, s2, t3 = S1b[0:1, tch, :], S2b[0:1, tch, :], T3b[0:1, tch, :]
                nc.vector.tensor_copy(s1, r1)
                nc.vector.tensor_copy(s2, r2)
                ln_rows(s1, s2, t3)
                a_ps = bc2_ps.tile([P, 512], F32, tag="bc")
                nc.tensor.matmul(a_ps, ones1, s2, start=True, stop=True)
                b_ps = bc2_ps.tile([P, 512], F32, tag="bc")
                nc.tensor.matmul(b_ps, ones1, s1, start=True, stop=True)
                for c in range(NC):
                    nc.vector.tensor_mul(h2T[:, c, tsl], x1T[:, c, tsl], a_ps)
                    nc.vector.tensor_add(h2T[:, c, tsl], h2T[:, c, tsl], b_ps)
                    if has_b:
                        nc.gpsimd.tensor_scalar_add(
                            h2T[:, c, tsl], h2T[:, c, tsl], b_sb[:, c:c + 1]
                        )

        # ---- phase 4: FFN + residual + store -----------------------------
        with ExitStack() as p4:
            f1_ps = p4.enter_context(
                tc.tile_pool(name="f1_ps", bufs=3, space="PSUM")
            )
            f2_ps = p4.enter_context(
                tc.tile_pool(name="f2_ps", bufs=2, space="PSUM")
            )
            fpool = p4.enter_context(tc.tile_pool(name="fpool", bufs=18))
            opool = p4.enter_context(tc.tile_pool(name="opool", bufs=6))
            for tch in range(2):
                tsl = slice(tch * 512, (tch + 1) * 512)
                ff1 = []
                for ht in range(NH2):
                    ps = f1_ps.tile([P, 512], F32, tag="f1")
                    for c in range(NC):
                        nc.tensor.matmul(
                            ps,
                            w1_sb[:, c, ht * P:(ht + 1) * P],
                            h2T[:, c, tsl],
                            start=(c == 0), stop=(c == NC - 1),
                        )
                    f1s = fpool.tile([P, 512], BF16, tag="f1s")
                    nc.scalar.activation(
                        out=f1s, in_=ps,
                        func=mybir.ActivationFunctionType.Relu,
                        bias=b1_sb[:, ht:ht + 1], scale=1.0,
                    )
                    ff1.append(f1s)
                for tbl in range(4):
                    tb = tch * 4 + tbl
                    ps2 = f2_ps.tile([P, D], F32, tag="f2")
                    for ht in range(NH2):
                        nc.tensor.matmul(
                            ps2,
                            ff1[ht][:, tbl * P:(tbl + 1) * P],
                            w2_sb[:, ht, :],
                            start=(ht == 0), stop=(ht == NH2 - 1),
                        )
                    orow = opool.tile([P, D], F32, tag="or")
                    nc.vector.tensor_add(orow, ps2, x1row[:, tb, :])
                    if has_b2:
                        nc.vector.tensor_add(orow, orow, b2_bc)
                    nc.sync.dma_start(
                        out=out[tb * P:(tb + 1) * P, :], in_=orow
                    )
    nc.compile()
    return nc


def _make_masks(par: int) -> np.ndarray:
    """[128, NT*128] multiplicative masks for the first suffix block."""
    m = np.zeros((NT, P, P), np.float32)
    for k in range(NT):
        g = 2 * JMIN[k] + par
        t_glob = g * P + np.arange(P)[None, :]
        s_glob = k * P + np.arange(P)[:, None]
        m[k] = (t_glob >= s_glob).astype(np.float32)
    return np.ascontiguousarray(
        m.transpose(1, 0, 2).reshape(P, NT * P).astype(ml_dtypes.bfloat16)
    )


def _chunk_rows(w: np.ndarray) -> np.ndarray:
    """[D_in, N] -> [128, (D_in/128)*N] with [p, c*N+n] = w[c*128+p, n]."""
    din, n = w.shape
    return np.ascontiguousarray(
        w.reshape(din // P, P, n).transpose(1, 0, 2).reshape(P, (din // P) * n)
    )


def _prep(inputs):
    f32 = lambda a: np.ascontiguousarray(np.asarray(a, dtype=np.float32))
    bf = lambda a: np.ascontiguousarray(
        np.asarray(a, dtype=np.float32).astype(ml_dtypes.bfloat16)
    )
    x = f32(inputs["x"])
    g = f32(inputs["ln1_g"])
    # [H, D, DH] -> [D, H*DH]; fold ln1_g into the input rows
    fold = lambda w: g[:, None] * np.asarray(w, np.float32).transpose(1, 0, 2).reshape(D, D)
    common = {
        "wq": bf(_chunk_rows(fold(inputs["Wq"]))),
        "wk": bf(_chunk_rows(fold(inputs["Wk"]))),
        "wv": bf(_chunk_rows(fold(inputs["Wv"]))),
        "wp": bf(_chunk_rows(f32(inputs["Wproj"]))),
        "w1": bf(_chunk_rows(g[:, None] * f32(inputs["W1"]))),
        "w2": bf(_chunk_rows(f32(inputs["W2"]))),
        "bvec": f32(inputs["ln1_b"]),
        "bpro": f32(inputs["bproj"]),
        "b1v": f32(inputs["b1"]),
        "b2v": f32(inputs["b2"]),
    }
    masks = [_make_masks(0), _make_masks(1)]
    in_maps = []
    for c in range(8):
        b, p = c // 2, c % 2
        xb = x[b]                                   # [T, D]
        xo_rows = np.ascontiguousarray(
            xb.reshape(NT, P, D)[p::2].reshape(TQ, D)
        )
        in_maps.append(dict(
            common,
            xT=bf(_chunk_rows(np.ascontiguousarray(xb.T))),
            xoT=bf(_chunk_rows(np.ascontiguousarray(xo_rows.T))),
            xo=np.ascontiguousarray(
                xo_rows.reshape(NQ, P, D).transpose(1, 0, 2).reshape(P, NQ * D)
            ),
            masks=masks[p],
        ))
    return in_maps


def _flags(inputs):
    nz = lambda k: bool(np.any(np.asarray(inputs[k], np.float32) != 0.0))
    return (nz("ln1_b"), nz("bproj"), nz("b2"))


def _run(inputs, trace=False):
    key = _flags(inputs)
    if key not in _CACHED:
        _CACHED[key] = _build_nc(*key)
    nc = _CACHED[key]
    in_maps = _prep(inputs)
    res = run_bass_kernel_spmd(nc, in_maps, core_ids=list(range(8)), trace=trace)
    out = np.empty((B, T, D), np.float32)
    for c in range(8):
        b, p = c // 2, c % 2
        out[b].reshape(NT, P, D)[p::2] = res.results[c]["out"].reshape(NQ, P, D)
    return out, res


def kernel(**inputs) -> np.ndarray:
    out, _ = _run(inputs, trace=False)
    return out


# revision 8
# speedup vs baseline: 1.4339x; 1.2939x over previous
"""Trainium2 Bass kernel for a dense pre-norm transformer block.

Reference semantics (B=4, T=2048, D=512, H=8, DH=64, fp32):
    h  = LN(x; g, b)
    q,k,v = per-head projections of h
    att = causal softmax(q k^T / sqrt(D))
    x1 = x + (att v) @ Wproj + bproj          (heads concatenated)
    h2 = LN(x1; g, b)                         (same LN params, faithful to source)
    out = x1 + relu(h2 @ W1 + b1) @ W2 + b2

Sharding: 8 cores = 4 batches x 2 parities. Core (b, p) owns the 8
row-blocks {p, p+2, ..., p+14} (128 rows each) of batch b. Causal key
extents are rounded up to 256 so even/odd block sets see identical
work -> one uniform SPMD program, no collectives. Exact causality is
restored with multiplicative 0/1 masks on the exp() values (host
provides per-parity masks and own-column slices).

v2 design (vs the first working version):
  - All large tensors are host-permuted so every DMA is one contiguous
    multi-KB descriptor per partition (128 descriptors per tensor).
  - No DRAM round-trips for LN stats or softmax denominators:
      * LN stats are computed in the transposed domain with ones-column
        matmuls (sum and sum-of-squares rows in PSUM), finished with a
        few single-partition row ops, and re-broadcast across partitions
        with rank-1 PE matmuls into PSUM (h = x*A + B with A = rstd,
        B = -mu*rstd; ln1_g is folded into Wq/Wk/Wv/W1 rows on the host).
      * softmax denominators (the free extra row from the ones-column
        appended to v) get reciprocal_approx_fast + rank-1 PE broadcast.
  - Diagonal-block causal masks are multiplied on the otherwise-idle
    GPSIMD engine; exp stays on the scalar engine (the attention-phase
    floor: ~9.4M exps at 1 elem/cycle/lane).
  - LN applies / qkv are chunk-pipelined so the PE starts ~40us earlier.
"""

import sys

sys.path.insert(0, "/opt/trn_rl_repo")

import numpy as np
import ml_dtypes
from contextlib import ExitStack

import concourse.bass as bass
import concourse.bacc as bacc
import concourse.mybir as mybir
import concourse.tile as tile
from concourse.bass_utils import run_bass_kernel_spmd

B, T, D, H = 4, 2048, 512, 8
DH = D // H            # 64
HID = 4 * D            # 2048
P = 128                # partitions
NT = T // P            # 16 row blocks over full T
NQ = 8                 # own row blocks per core
TQ = NQ * P            # 1024 own rows per core
NC = D // P            # 4 feature chunks
NH2 = HID // P         # 16
EPS = 1e-5
SCALE = D ** -0.5
F32 = mybir.dt.float32
BF16 = mybir.dt.bfloat16
AOP = mybir.AluOpType

# first own-block (local index) attending key-block k; extents rounded to 256
JMIN = [k // 2 for k in range(16)]

_CACHED = {}


def _build_nc(has_b: bool, has_bpro: bool, has_b2: bool):
    nc = bacc.Bacc()

    # host-permuted contiguous inputs: [p, c*N+n] = tensor[c*128+p, n]
    xT = nc.dram_tensor("xT", [P, NC * T], BF16, kind="ExternalInput")
    xoT = nc.dram_tensor("xoT", [P, NC * TQ], BF16, kind="ExternalInput")
    xo = nc.dram_tensor("xo", [P, NQ * D], F32, kind="ExternalInput")
    wq = nc.dram_tensor("wq", [P, NC * D], BF16, kind="ExternalInput")
    wk = nc.dram_tensor("wk", [P, NC * D], BF16, kind="ExternalInput")
    wv = nc.dram_tensor("wv", [P, NC * D], BF16, kind="ExternalInput")
    wp = nc.dram_tensor("wp", [P, NC * D], BF16, kind="ExternalInput")
    w1 = nc.dram_tensor("w1", [P, NC * HID], BF16, kind="ExternalInput")
    w2 = nc.dram_tensor("w2", [P, NH2 * D], BF16, kind="ExternalInput")
    masks = nc.dram_tensor("masks", [P, NT * P], BF16, kind="ExternalInput")
    bvec = nc.dram_tensor("bvec", [D], F32, kind="ExternalInput")
    bpro = nc.dram_tensor("bpro", [D], F32, kind="ExternalInput")
    b1v = nc.dram_tensor("b1v", [HID], F32, kind="ExternalInput")
    b2v = nc.dram_tensor("b2v", [D], F32, kind="ExternalInput")
    out = nc.dram_tensor("out", [TQ, D], F32, kind="ExternalOutput")

    with ExitStack() as ctx:
        tc = ctx.enter_context(tile.TileContext(nc))
        consts = ctx.enter_context(tc.tile_pool(name="consts", bufs=1))
        acts = ctx.enter_context(tc.tile_pool(name="acts", bufs=1))

        # ---- input x first (DMA queue priority), then weights -------------
        xT_pre = consts.tile([P, NC, T], BF16)
        nc.sync.dma_start(out=xT_pre, in_=xT[:].rearrange("p (c n) -> p c n", c=NC))
        xoT_sb = consts.tile([P, NC, TQ], BF16)
        nc.sync.dma_start(out=xoT_sb, in_=xoT[:].rearrange("p (c n) -> p c n", c=NC))
        wq_sb = consts.tile([P, NC, D], BF16)
        nc.sync.dma_start(out=wq_sb, in_=wq[:].rearrange("p (c n) -> p c n", c=NC))
        wk_sb = consts.tile([P, NC, D], BF16)
        nc.sync.dma_start(out=wk_sb, in_=wk[:].rearrange("p (c n) -> p c n", c=NC))
        wv_sb = consts.tile([P, NC, D], BF16)
        nc.sync.dma_start(out=wv_sb, in_=wv[:].rearrange("p (c n) -> p c n", c=NC))
        masks_sb = consts.tile([P, NT, P], BF16)
        nc.sync.dma_start(out=masks_sb, in_=masks[:].rearrange("p (k c) -> p k c", k=NT))
        wp_sb = consts.tile([P, NC, D], BF16)
        nc.sync.dma_start(out=wp_sb, in_=wp[:].rearrange("p (c n) -> p c n", c=NC))
        w1_sb = consts.tile([P, NC, HID], BF16)
        nc.sync.dma_start(out=w1_sb, in_=w1[:].rearrange("p (c n) -> p c n", c=NC))
        w2_sb = consts.tile([P, NH2, D], BF16)
        nc.sync.dma_start(out=w2_sb, in_=w2[:].rearrange("p (c n) -> p c n", c=NH2))
        b1_sb = consts.tile([P, NH2], F32)
        nc.sync.dma_start(out=b1_sb, in_=b1v[:].rearrange("(c p) -> p c", p=P))

        onesc = consts.tile([P, 1], BF16)
        nc.vector.memset(onesc, 1.0)
        ones1 = consts.tile([1, P], F32)
        nc.vector.memset(ones1, 1.0)
        ones1b = consts.tile([1, P], BF16)
        nc.vector.memset(ones1b, 1.0)
        eps1 = consts.tile([1, 1], F32)
        nc.vector.memset(eps1, EPS)
        if has_b:
            b_sb = consts.tile([P, NC], F32)
            nc.sync.dma_start(out=b_sb, in_=bvec[:].rearrange("(c p) -> p c", p=P))
        if has_bpro:
            bpro_sb = consts.tile([P, NC], F32)
            nc.sync.dma_start(out=bpro_sb, in_=bpro[:].rearrange("(c p) -> p c", p=P))
            bpro_bc = consts.tile([P, D], F32)
            nc.gpsimd.dma_start(out=bpro_bc, in_=bpro[:].partition_broadcast(P))
        if has_b2:
            b2_bc = consts.tile([P, D], F32)
            nc.gpsimd.dma_start(out=b2_bc, in_=b2v[:].partition_broadcast(P))

        # ---- persistent activations --------------------------------------
        qT = acts.tile([P, NC, TQ], BF16)        # own columns, compact
        kT = acts.tile([P, NC, T], BF16)
        v_aug = acts.tile([P, NT, H, DH + 1], BF16)   # v + ones column
        o_catT = acts.tile([P, NC, TQ], BF16)
        x1T = acts.tile([P, NC, TQ], BF16)
        h2T = acts.tile([P, NC, TQ], BF16)
        x1row = acts.tile([P, NQ, D], F32)

        def ln_rows(r1, r2, s1, s2, t3, a_bf, b_bf):
            """Finish LN stats rows from PSUM sums; a_bf=rstd, b_bf=-mu*rstd."""
            nc.scalar.mul(s1, r1, 1.0 / D)                    # mu
            nc.scalar.mul(s2, r2, 1.0 / D)                    # E[x^2]
            nc.vector.tensor_mul(t3, s1, s1)                  # mu^2
            nc.vector.tensor_sub(s2, s2, t3)                  # var
            nc.scalar.activation(
                out=s2, in_=s2,
                func=mybir.ActivationFunctionType.Sqrt,
                bias=eps1, scale=1.0,
            )
            nc.vector.reciprocal_approx_fast(out=s2, in_=s2)  # A = rstd
            nc.scalar.copy(a_bf, s2)
            nc.vector.scalar_tensor_tensor(                   # B = -mu*A
                out=b_bf, in0=s1, scalar=-1.0, in1=s2,
                op0=AOP.mult, op1=AOP.mult,
            )

        # ---- phase 1: LN1 (transposed-domain stats) + q/k/v --------------
        with ExitStack() as p1:
            lnp = p1.enter_context(tc.tile_pool(name="lnp", bufs=1))
            sqp = p1.enter_context(tc.tile_pool(name="sqp", bufs=2))
            rows_ps = p1.enter_context(
                tc.tile_pool(name="rows_ps", bufs=2, space="PSUM")
            )
            bc_ps = p1.enter_context(
                tc.tile_pool(name="bc_ps", bufs=4, space="PSUM")
            )
            qkv_ps = p1.enter_context(
                tc.tile_pool(name="qkv_ps", bufs=2, space="PSUM")
            )

            xT_sb = xT_pre
            hT = lnp.tile([P, NC, T], BF16)
            hoT = h2T  # reuse (h2T is not live until phase 3)
            S1 = lnp.tile([1, 2, 512], F32)    # mu scratch (ping-pong)
            S2 = lnp.tile([1, 2, 512], F32)    # var/rstd scratch
            T3 = lnp.tile([1, 2, 512], BF16)   # mu^2 scratch
            A_bf = lnp.tile([1, 6, 512], BF16)  # rstd rows: 4 full + 2 own
            B_bf = lnp.tile([1, 6, 512], BF16)  # -mu*rstd rows

            # own-column stats + apply + qT (own chunks are compact)
            def emit_own_chunk(qch):
                tsl = slice(qch * 512, (qch + 1) * 512)
                sq = sqp.tile([P, NC, 512], BF16, tag="sq")
                nc.vector.tensor_mul(sq, xoT_sb[:, :, tsl], xoT_sb[:, :, tsl])
                r1 = rows_ps.tile([1, 512], F32, tag="r")
                for c in range(NC):
                    nc.tensor.matmul(
                        r1, onesc, xoT_sb[:, c, tsl],
                        start=(c == 0), stop=(c == NC - 1),
                    )
                r2 = rows_ps.tile([1, 512], F32, tag="r")
                for c in range(NC):
                    nc.tensor.matmul(
                        r2, onesc, sq[:, c, :],
                        start=(c == 0), stop=(c == NC - 1),
                    )
                a_bf, b_bf = A_bf[0:1, 4 + qch, :], B_bf[0:1, 4 + qch, :]
                pp = qch % 2
                ln_rows(r1, r2, S1[0:1, pp, :], S2[0:1, pp, :], T3[0:1, pp, :],
                        a_bf, b_bf)
                a_ps = bc_ps.tile([P, 512], F32, tag="bc")
                nc.tensor.matmul(a_ps, ones1b, a_bf, start=True, stop=True)
                b_ps = bc_ps.tile([P, 512], F32, tag="bc")
                nc.tensor.matmul(b_ps, ones1b, b_bf, start=True, stop=True)
                for c in range(NC):
                    nc.vector.tensor_mul(hoT[:, c, tsl], xoT_sb[:, c, tsl], a_ps)
                    nc.vector.tensor_add(hoT[:, c, tsl], hoT[:, c, tsl], b_ps)
                    if has_b:
                        nc.gpsimd.tensor_scalar_add(
                            hoT[:, c, tsl], hoT[:, c, tsl], b_sb[:, c:c + 1]
                        )
                for pair in range(4):
                    ps_q = qkv_ps.tile([P, 512], F32, tag="ps")
                    for c in range(NC):
                        nc.tensor.matmul(
                            ps_q,
                            wq_sb[:, c, pair * P:(pair + 1) * P],
                            hoT[:, c, tsl],
                            start=(c == 0), stop=(c == NC - 1),
                        )
                    nc.any.tensor_copy(qT[:, pair, tsl], ps_q)

            emit_own_chunk(0)
            emit_own_chunk(1)

            for tch in range(4):
                tsl = slice(tch * 512, (tch + 1) * 512)
                sq = sqp.tile([P, NC, 512], BF16, tag="sq")
                nc.vector.tensor_mul(sq, xT_sb[:, :, tsl], xT_sb[:, :, tsl])
                r1 = rows_ps.tile([1, 512], F32, tag="r")
                for c in range(NC):
                    nc.tensor.matmul(
                        r1, onesc, xT_sb[:, c, tsl],
                        start=(c == 0), stop=(c == NC - 1),
                    )
                r2 = rows_ps.tile([1, 512], F32, tag="r")
                for c in range(NC):
                    nc.tensor.matmul(
                        r2, onesc, sq[:, c, :],
                        start=(c == 0), stop=(c == NC - 1),
                    )
                a_bf, b_bf = A_bf[0:1, tch, :], B_bf[0:1, tch, :]
                pp = tch % 2
                ln_rows(r1, r2, S1[0:1, pp, :], S2[0:1, pp, :], T3[0:1, pp, :],
                        a_bf, b_bf)
                a_ps = bc_ps.tile([P, 512], F32, tag="bc")
                nc.tensor.matmul(a_ps, ones1b, a_bf, start=True, stop=True)
                b_ps = bc_ps.tile([P, 512], F32, tag="bc")
                nc.tensor.matmul(b_ps, ones1b, b_bf, start=True, stop=True)
                for c in range(NC):
                    nc.vector.tensor_mul(hT[:, c, tsl], xT_sb[:, c, tsl], a_ps)
                    nc.vector.tensor_add(hT[:, c, tsl], hT[:, c, tsl], b_ps)
                    if has_b:
                        nc.gpsimd.tensor_scalar_add(
                            hT[:, c, tsl], hT[:, c, tsl], b_sb[:, c:c + 1]
                        )
                # kT / v for this chunk
                for pair in range(4):
                    ps_k = qkv_ps.tile([P, 512], F32, tag="ps")
                    for c in range(NC):
                        nc.tensor.matmul(
                            ps_k,
                            wk_sb[:, c, pair * P:(pair + 1) * P],
                            hT[:, c, tsl],
                            start=(c == 0), stop=(c == NC - 1),
                        )
                    nc.any.tensor_copy(kT[:, pair, tsl], ps_k)
                for st in range(4 * tch, 4 * tch + 4):
                    ps_v = qkv_ps.tile([P, 512], F32, tag="ps")
                    for c in range(NC):
                        nc.tensor.matmul(
                            ps_v,
                            hT[:, c, st * P:(st + 1) * P],
                            wv_sb[:, c, :],
                            start=(c == 0), stop=(c == NC - 1),
                        )
                    nc.any.tensor_copy(
                        v_aug[:, st, :, 0:DH],
                        ps_v.rearrange("p (h e) -> p h e", h=H),
                    )
                    nc.vector.memset(v_aug[:, st, :, DH:DH + 1], 1.0)

        # ---- phase 2: attention ------------------------------------------
        prs = [slice(0, DH), slice(DH, 2 * DH)]
        with ExitStack() as p2:
            sc_ps = p2.enter_context(
                tc.tile_pool(name="sc_ps", bufs=2, space="PSUM")
            )
            av_ps = p2.enter_context(
                tc.tile_pool(name="av_ps", bufs=4, space="PSUM")
            )
            epool = p2.enter_context(tc.tile_pool(name="epool", bufs=6))
            rpool = p2.enter_context(tc.tile_pool(name="rpool", bufs=4))
            dpool = p2.enter_context(tc.tile_pool(name="dpool", bufs=2))
            for pair in range(4):
                oc00 = av_ps.tile([P, 512], F32, tag="oc")
                oc01 = av_ps.tile([P, 512], F32, tag="oc")
                oc10 = av_ps.tile([P, 512], F32, tag="oc")
                oc11 = av_ps.tile([P, 512], F32, tag="oc")
                ocs = [[oc00, oc01], [oc10, oc11]]   # [half][chunk]
                for k in range(NT):
                    ss = P * JMIN[k]
                    L = TQ - ss
                    if L > 512:
                        sco0 = sc_ps.tile([P, 1024], F32, tag="sc")
                        sco1 = sc_ps.tile([P, 1024], F32, tag="sc")
                        scos, offs = [sco0, sco1], [0, 0]
                    else:
                        # both halves packed in one tile -> 2 k-blocks in flight
                        sco = sc_ps.tile([P, 1024], F32, tag="sc")
                        scos, offs = [sco, sco], [0, 512]
                    for half in range(2):
                        for n0 in range(0, L, 512):
                            nn = min(512, L - n0)
                            o0 = offs[half] + n0
                            nc.tensor.matmul(
                                scos[half][:, o0:o0 + nn],
                                kT[prs[half], pair, k * P:(k + 1) * P],
                                qT[prs[half], pair, ss + n0:ss + n0 + nn],
                                start=True, stop=True,
                            )
                    for half in range(2):
                        h = 2 * pair + half
                        oc0, oc1 = ocs[half]
                        ex = epool.tile([P, 1024], BF16, tag="ex")
                        nc.scalar.activation(
                            out=ex[:, 0:L],
                            in_=scos[half][:, offs[half]:offs[half] + L],
                            func=mybir.ActivationFunctionType.Exp,
                            scale=SCALE,
                        )
                        nc.gpsimd.tensor_mul(
                            ex[:, 0:P], ex[:, 0:P], masks_sb[:, k, :]
                        )
                        lhs_v = v_aug[:, k, h, :]
                        if ss < 512:
                            nc.tensor.matmul(
                                oc0[0:DH + 1, ss:512],
                                lhs_v,
                                ex[:, 0:512 - ss],
                                start=(k == 0), stop=(k == 7),
                            )
                            nc.tensor.matmul(
                                oc1[0:DH + 1, :],
                                lhs_v,
                                ex[:, 512 - ss:L],
                                start=(k == 0), stop=(k == 15),
                            )
                        else:
                            nc.tensor.matmul(
                                oc1[0:DH + 1, ss - 512:512],
                                lhs_v,
                                ex[:, 0:L],
                                start=False, stop=(k == 15),
                            )
                # denominators -> 1/den -> rank-1 PE broadcast -> normalize
                den = dpool.tile([1, 2, TQ], F32, tag="den")
                raws = []
                for half in range(2):
                    oc0, oc1 = ocs[half]
                    raw = rpool.tile([DH, TQ], BF16, tag="raw")
                    nc.vector.tensor_copy(raw[:, 0:512], oc0[0:DH, :])
                    nc.vector.tensor_copy(raw[:, 512:TQ], oc1[0:DH, :])
                    nc.vector.tensor_copy(den[0:1, half, 0:512], oc0[DH:DH + 1, :])
                    nc.vector.tensor_copy(den[0:1, half, 512:TQ], oc1[DH:DH + 1, :])
                    raws.append(raw)
                nc.vector.reciprocal_approx_fast(out=den, in_=den)
                ib0 = av_ps.tile([P, 512], F32, tag="oc")
                ib1 = av_ps.tile([P, 512], F32, tag="oc")
                ibs = [ib0, ib1]
                for half in range(2):
                    for ch in range(2):
                        nc.tensor.matmul(
                            ibs[ch][64 * half:64 * half + DH, :],
                            ones1[0:1, 0:DH],
                            den[0:1, half, ch * 512:(ch + 1) * 512],
                            start=True, stop=True,
                        )
                for half in range(2):
                    for ch in range(2):
                        nc.vector.tensor_mul(
                            o_catT[prs[half], pair, ch * 512:(ch + 1) * 512],
                            raws[half][:, ch * 512:(ch + 1) * 512],
                            ibs[ch][64 * half:64 * half + DH, :],
                        )

        # ---- phase 3: proj (both orientations), residual, LN2 -------------
        with ExitStack() as p3:
            pr_ps = p3.enter_context(
                tc.tile_pool(name="pr_ps", bufs=2, space="PSUM")
            )
            prT_ps = p3.enter_context(
                tc.tile_pool(name="prT_ps", bufs=2, space="PSUM")
            )
            rows2_ps = p3.enter_context(
                tc.tile_pool(name="rows2_ps", bufs=2, space="PSUM")
            )
            bc2_ps = p3.enter_context(
                tc.tile_pool(name="bc2_ps", bufs=2, space="PSUM")
            )
            xopool = p3.enter_context(tc.tile_pool(name="xopool", bufs=4))
            sq2p = p3.enter_context(tc.tile_pool(name="sq2p", bufs=2))
            ln2p = p3.enter_context(tc.tile_pool(name="ln2p", bufs=1))

            # row-major proj + residual
            for tb in range(NQ):
                xo_t = xopool.tile([P, D], F32, tag="xo")
                nc.sync.dma_start(
                    out=xo_t,
                    in_=xo[:].rearrange("p (a b) -> p a b", a=NQ)[:, tb, :],
                )
                ps = pr_ps.tile([P, D], F32, tag="pp")
                for pair in range(4):
                    nc.tensor.matmul(
                        ps,
                        o_catT[:, pair, tb * P:(tb + 1) * P],
                        wp_sb[:, pair, :],
                        start=(pair == 0), stop=(pair == 3),
                    )
                nc.vector.tensor_add(x1row[:, tb, :], ps, xo_t)
                if has_bpro:
                    nc.vector.tensor_add(
                        x1row[:, tb, :], x1row[:, tb, :], bpro_bc
                    )

            # transposed proj: x1T = xoT + Wproj^T @ o_catT (+ bproj)
            for dt in range(NC):
                for tch in range(2):
                    sl = slice(tch * 512, (tch + 1) * 512)
                    psT = prT_ps.tile([P, 512], F32, tag="pt")
                    for pair in range(4):
                        nc.tensor.matmul(
                            psT,
                            wp_sb[:, pair, dt * P:(dt + 1) * P],
                            o_catT[:, pair, sl],
                            start=(pair == 0), stop=(pair == 3),
                        )
                    nc.vector.tensor_add(x1T[:, dt, sl], psT, xoT_sb[:, dt, sl])
                    if has_bpro:
                        nc.vector.tensor_scalar_add(
                            x1T[:, dt, sl], x1T[:, dt, sl], bpro_sb[:, dt:dt + 1]
                        )

            # LN2 stats from x1T + apply -> h2T
            S1b = ln2p.tile([1, 2, 512], F32)
            S2b = ln2p.tile([1, 2, 512], F32)
            T3b = ln2p.tile([1, 2, 512], BF16)
            A2_bf = ln2p.tile([1, 2, 512], BF16)
            B2_bf = ln2p.tile([1, 2, 512], BF16)
            for tch in range(2):
                tsl = slice(tch * 512, (tch + 1) * 512)
                sq2 = sq2p.tile([P, NC, 512], BF16, tag="sq2")
                nc.vector.tensor_mul(sq2, x1T[:, :, tsl], x1T[:, :, tsl])
                r1 = rows2_ps.tile([1, 512], F32, tag="r")
                for c in range(NC):
                    nc.tensor.matmul(
                        r1, onesc, x1T[:, c, tsl],
                        start=(c == 0), stop=(c == NC - 1),
                    )
                r2 = rows2_ps.tile([1, 512], F32, tag="r")
                for c in range(NC):
                    nc.tensor.matmul(
                        r2, onesc, sq2[:, c, :],
                        start=(c == 0), stop=(c == NC - 1),
                    )
                a_bf, b_bf = A2_bf[0:1, tch, :], B2_bf[0:1, tch, :]
                ln_rows(r1, r2, S1b[0:1, tch, :], S2b[0:1, tch, :],
                        T3b[0:1, tch, :], a_bf, b_bf)
                a_ps = bc2_ps.tile([P, 512], F32, tag="bc")
                nc.tensor.matmul(a_ps, ones1b, a_bf, start=True, stop=True)
                b_ps = bc2_ps.tile([P, 512], F32, tag="bc")
                nc.tensor.matmul(b_ps, ones1b, b_bf, start=True, stop=True)
                for c in range(NC):
                    nc.vector.tensor_mul(h2T[:, c, tsl], x1T[:, c, tsl], a_ps)
                    nc.vector.tensor_add(h2T[:, c, tsl], h2T[:, c, tsl], b_ps)
                    if has_b:
                        nc.gpsimd.tensor_scalar_add(
                            h2T[:, c, tsl], h2T[:, c, tsl], b_sb[:, c:c + 1]
                        )

        # ---- phase 4: FFN + residual + store -----------------------------
        with ExitStack() as p4:
            f1_ps = p4.enter_context(
                tc.tile_pool(name="f1_ps", bufs=4, space="PSUM")
            )
            f2_ps = p4.enter_context(
                tc.tile_pool(name="f2_ps", bufs=2, space="PSUM")
            )
            fpool = p4.enter_context(tc.tile_pool(name="fpool", bufs=18))
            opool = p4.enter_context(tc.tile_pool(name="opool", bufs=6))
            for tch in range(2):
                tsl = slice(tch * 512, (tch + 1) * 512)
                ff1 = []
                for ht in range(NH2):
                    ps = f1_ps.tile([P, 512], F32, tag="f1")
                    for c in range(NC):
                        nc.tensor.matmul(
                            ps,
                            w1_sb[:, c, ht * P:(ht + 1) * P],
                            h2T[:, c, tsl],
                            start=(c == 0), stop=(c == NC - 1),
                        )
                    f1s = fpool.tile([P, 512], BF16, tag="f1s")
                    nc.scalar.activation(
                        out=f1s, in_=ps,
                        func=mybir.ActivationFunctionType.Relu,
                        bias=b1_sb[:, ht:ht + 1], scale=1.0,
                    )
                    ff1.append(f1s)
                for tbl in range(4):
                    tb = tch * 4 + tbl
                    ps2 = f2_ps.tile([P, D], F32, tag="f2")
                    for ht in range(NH2):
                        nc.tensor.matmul(
                            ps2,
                            ff1[ht][:, tbl * P:(tbl + 1) * P],
                            w2_sb[:, ht, :],
                            start=(ht == 0), stop=(ht == NH2 - 1),
                        )
                    orow = opool.tile([P, D], F32, tag="or")
                    nc.vector.tensor_add(orow, ps2, x1row[:, tb, :])
                    if has_b2:
                        nc.vector.tensor_add(orow, orow, b2_bc)
                    nc.sync.dma_start(
                        out=out[tb * P:(tb + 1) * P, :], in_=orow
                    )
    nc.compile()
    return nc


def _make_masks(par: int) -> np.ndarray:
    """[128, NT*128] multiplicative masks for the first suffix block."""
    m = np.zeros((NT, P, P), np.float32)
    for k in range(NT):
        g = 2 * JMIN[k] + par
        t_glob = g * P + np.arange(P)[None, :]
        s_glob = k * P + np.arange(P)[:, None]
        m[k] = (t_glob >= s_glob).astype(np.float32)
    return np.ascontiguousarray(
        m.transpose(1, 0, 2).reshape(P, NT * P).astype(ml_dtypes.bfloat16)
    )


def _chunk_rows(w: np.ndarray) -> np.ndarray:
    """[D_in, N] -> [128, (D_in/128)*N] with [p, c*N+n] = w[c*128+p, n]."""
    din, n = w.shape
    return np.ascontiguousarray(
        w.reshape(din // P, P, n).transpose(1, 0, 2).reshape(P, (din // P) * n)
    )


def _prep(inputs):
    f32 = lambda a: np.ascontiguousarray(np.asarray(a, dtype=np.float32))
    bf = lambda a: np.ascontiguousarray(
        np.asarray(a, dtype=np.float32).astype(ml_dtypes.bfloat16)
    )
    x = f32(inputs["x"])
    g = f32(inputs["ln1_g"])
    # [H, D, DH] -> [D, H*DH]; fold ln1_g into the input rows
    fold = lambda w: g[:, None] * np.asarray(w, np.float32).transpose(1, 0, 2).reshape(D, D)
    common = {
        "wq": bf(_chunk_rows(fold(inputs["Wq"]))),
        "wk": bf(_chunk_rows(fold(inputs["Wk"]))),
        "wv": bf(_chunk_rows(fold(inputs["Wv"]))),
        "wp": bf(_chunk_rows(f32(inputs["Wproj"]))),
        "w1": bf(_chunk_rows(g[:, None] * f32(inputs["W1"]))),
        "w2": bf(_chunk_rows(f32(inputs["W2"]))),
        "bvec": f32(inputs["ln1_b"]),
        "bpro": f32(inputs["bproj"]),
        "b1v": f32(inputs["b1"]),
        "b2v": f32(inputs["b2"]),
    }
    masks = [_make_masks(0), _make_masks(1)]
    in_maps = []
    for c in range(8):
        b, p = c // 2, c % 2
        xb = x[b]                                   # [T, D]
        xo_rows = np.ascontiguousarray(
            xb.reshape(NT, P, D)[p::2].reshape(TQ, D)
        )
        in_maps.append(dict(
            common,
            xT=bf(_chunk_rows(np.ascontiguousarray(xb.T))),
            xoT=bf(_chunk_rows(np.ascontiguousarray(xo_rows.T))),
            xo=np.ascontiguousarray(
                xo_rows.reshape(NQ, P, D).transpose(1, 0, 2).reshape(P, NQ * D)
            ),
            masks=masks[p],
        ))
    return in_maps


def _flags(inputs):
    nz = lambda k: bool(np.any(np.asarray(inputs[k], np.float32) != 0.0))
    return (nz("ln1_b"), nz("bproj"), nz("b2"))


def _run(inputs, trace=False):
    key = _flags(inputs)
    if key not in _CACHED:
        _CACHED[key] = _build_nc(*key)
    nc = _CACHED[key]
    in_maps = _prep(inputs)
    res = run_bass_kernel_spmd(nc, in_maps, core_ids=list(range(8)), trace=trace)
    out = np.empty((B, T, D), np.float32)
    for c in range(8):
        b, p = c // 2, c % 2
        out[b].reshape(NT, P, D)[p::2] = res.results[c]["out"].reshape(NQ, P, D)
    return out, res


def kernel(**inputs) -> np.ndarray:
    out, _ = _run(inputs, trace=False)
    return out
